# revision 13
# baseline (speedup 1.0000x reference)
"""MHSA (RoPE + causal softmax) Trainium2 Bass kernel.

Problem: x[4,2048,1024], Wq/Wk/Wv/Wo[1024,1024] fp32; 16 heads, d_k=64.

Sharding over the 8 NeuronCores: 4-way data-parallel over batch x 2-way
tensor-parallel over heads. core = 2*b + t handles batch b, heads
[t*8, t*8+8). Column-parallel Wq/Wk/Wv, row-parallel Wo; the two TP
partial outputs per batch are summed on the host (the gather step).

Device-side design (per core, all sizes hardcoded):
  - Host supplies x^T [1024,2048] so every matmul contraction dim (model
    dim d or k-positions) lands on SBUF partitions. float32r matmuls
    (fp32 bit layout, tf32-like multiply) run at ~1 cycle/row.
  - RoPE: interleaved even/odd pairs are pre-permuted in the Wq/Wk ROWS
    (host side) so each rotation partner lives 16 partitions away within
    a 32-partition quadrant; the rotation is then
        q' = q * cosT + stream_shuffle(q) * sinT
    with a single DVE stream-shuffle (swap 16-blocks) and sign baked
    into sinT. Scores are permutation-invariant so nothing downstream
    changes.
  - Attention per head pair (2 heads stacked on 128 partitions,
    auto row-tiled matmuls at base partitions 0/64):
       S^T[j] = K_j @ Q_I^T          (f32r, [128 kpos, <=512 q])
       P^T    = exp(S^T / 8)          (ACT, psum -> bf16 sbuf)
       causal: block-skip j>4I+3, narrow diagonal tiles, one [128,128]
       tri-mask multiply on the diagonal block
       O^T   += [V_j | 1]^T @ P^T     (bf16, gives output AND softmax
                                       denominator in row 64 - no
                                       max-subtraction softmax, safe for
                                       these magnitudes)
       mh^T   = O^T[0:64] * bcast(1/O^T[64])
  - Output projection back in [s, d] layout (lhsT = mh^T tiles) so the
    DRAM write is contiguous; host sums the two TP partials per batch.
"""
import numpy as np
import ml_dtypes

import concourse.bass as bass
from concourse import bacc
import concourse.tile as tile
import concourse.mybir as mybir
from concourse.bass_utils import run_bass_kernel_spmd

B, S, D = 4, 2048, 1024
HEADS, DK = 16, 64
THETA = 10000.0
TP, DP = 2, 4
HL = HEADS // TP            # 8 local heads per core
DL = HL * DK                # 512 local projection width
P = 128
SB = 512                    # q super-tile width
NSB = S // SB               # 4 q super-tiles (I)
NST = S // P                # 16 k-tiles (j)
NDC = D // P                # 8 contraction chunks over model dim
NPAIR = HL // 2             # 4 local head pairs

f32 = mybir.dt.float32
f32r = mybir.dt.float32r
bf16 = mybir.dt.bfloat16
SWAP16 = [(i + 16) % 32 for i in range(32)]


def _build():
    ALU = mybir.AluOpType
    FX = mybir.ActivationFunctionType
    nc = bacc.Bacc(None, target_bir_lowering=False)

    xT = nc.dram_tensor("xT", [D, S], bf16, kind="ExternalInput")
    wqkvT = nc.dram_tensor("wqkvT", [D, 3 * DL], bf16, kind="ExternalInput")
    woT = nc.dram_tensor("woT", [DL, D], bf16, kind="ExternalInput")
    cosf = nc.dram_tensor("cosf", [DK, S], f32, kind="ExternalInput")
    sinf = nc.dram_tensor("sinf", [DK, S], f32, kind="ExternalInput")
    maskt = nc.dram_tensor("maskt", [P, P], bf16, kind="ExternalInput")
    out = nc.dram_tensor("out", [S, D], f32, kind="ExternalOutput")

    xT_t = xT.rearrange("(dc p) s -> p dc s", p=P)        # [128, 8, 2048]
    wq_t = wqkvT.rearrange("(dc p) f -> p dc f", p=P)     # [128, 8, 1536]
    wo_t = woT.rearrange("(c p) f -> p c f", p=P)         # [128, 4, 1024]

    with tile.TileContext(nc) as tc:
        with (
            tc.tile_pool(name="wpool", bufs=1) as wpool,
            tc.tile_pool(name="kpool", bufs=1) as kpool,
            tc.tile_pool(name="xpool", bufs=2) as xpool,
            tc.tile_pool(name="qpool", bufs=2) as qpool,
            tc.tile_pool(name="tpool", bufs=2) as tpool,
            tc.tile_pool(name="ptpool", bufs=4) as ptpool,
            tc.tile_pool(name="mpool", bufs=2) as mpool,
            tc.tile_pool(name="spool", bufs=2) as spool,
            tc.tile_pool(name="opool", bufs=2) as opool,
            tc.tile_pool(name="ps_proj", bufs=2, space="PSUM") as ps_proj,
            tc.tile_pool(name="ps_s", bufs=2, space="PSUM") as ps_s,
            tc.tile_pool(name="ps_av", bufs=1, space="PSUM") as ps_av,
        ):
            # First q-block x chunks go out first so the PE can start within
            # a few us; bulky constants stream behind them.
            xt0 = xpool.tile([P, NDC, SB], bf16, tag="xt", name="xt0")
            w_sb = wpool.tile([P, NDC, 3 * DL], bf16)
            for dc in range(NDC):
                eng = nc.sync if dc % 2 == 0 else nc.scalar
                eng.dma_start(xt0[:, dc, :], xT_t[:, dc, 0:SB])
                eng2 = nc.scalar if dc % 2 == 0 else nc.sync
                eng2.dma_start(w_sb[:, dc, 0:DL], wq_t[:, dc, 0:DL])
            for dc in range(NDC):
                eng = nc.sync if dc % 2 == 0 else nc.scalar
                eng.dma_start(w_sb[:, dc, DL : 3 * DL], wq_t[:, dc, DL : 3 * DL])
            cos_sb = wpool.tile([P, S], f32)
            sin_sb = wpool.tile([P, S], f32)
            nc.gpsimd.dma_start(cos_sb[0:DK, :], cosf[:])
            nc.gpsimd.dma_start(cos_sb[DK:P, :], cosf[:])
            nc.gpsimd.dma_start(sin_sb[0:DK, :], sinf[:])
            nc.gpsimd.dma_start(sin_sb[DK:P, :], sinf[:])
            mask_sb = wpool.tile([P, P], bf16)
            nc.gpsimd.dma_start(mask_sb[:], maskt[:])
            wo_sb = wpool.tile([P, DL // P, D], bf16)

            ktall = kpool.tile([P, NPAIR, S], bf16)
            v_sb = kpool.tile([P, NST, HL, 2 * DK], bf16)
            nc.gpsimd.memset(v_sb[:, :, :, DK : 2 * DK], 1.0)

            # PE warm-up: the first ~15us are DMA-bound; keep the tensor
            # engine active so the HAM clock gate reaches (and holds) 2.4GHz
            # before the real matmuls arrive.
            warm = wpool.tile([P, SB], bf16)
            nc.gpsimd.memset(warm[:], 0.0)
            warm_sink = wpool.tile([P, SB], f32)

            def emit_warm(n):
                pw = ps_s.tile([P, 2, SB], f32, tag="pss", name=f"warm{emit_warm.k}")
                emit_warm.k += 1
                for r in range(n):
                    nc.tensor.matmul(pw[:, 0, :], warm[:, 0:P], warm[:],
                                     start=True, stop=True)
                nc.vector.tensor_copy(warm_sink[:], pw[:, 0, :])
            emit_warm.k = 0
            emit_warm(16)

            def emit_outproj(Io, mh):
                # row-parallel partial output projection for q block Io
                for st in range(SB // P):
                    osb = opool.tile([P, D], f32, tag="osb", name=f"osb{Io}_{st}")
                    for oh in (0, 1):
                        pq = ps_proj.tile([P, SB], f32, tag="pp", name=f"pq{Io}_{st}_{oh}")
                        for c2 in range(NPAIR):
                            nc.tensor.matmul(
                                pq[:],
                                mh[:, c2, st * P : (st + 1) * P],
                                wo_sb[:, c2, oh * SB : (oh + 1) * SB],
                                start=(c2 == 0),
                                stop=(c2 == NPAIR - 1),
                            )
                        nc.vector.tensor_copy(osb[:, oh * SB : (oh + 1) * SB], pq[:])
                    nc.sync.dma_start(
                        out[(Io * 4 + st) * P : (Io * 4 + st + 1) * P, :], osb[:]
                    )

            mh_prev = None
            for I in range(NSB):
                scol = slice(I * SB, (I + 1) * SB)
                if I == 0:
                    xt = xt0
                else:
                    xt = xpool.tile([P, NDC, SB], bf16, tag="xt", name=f"xt{I}")
                    for dc in range(NDC):
                        eng = nc.sync if dc % 2 == 0 else nc.scalar
                        eng.dma_start(xt[:, dc, :], xT_t[:, dc, scol])

                # ---- Q/K projections + RoPE (f-chunk = head pair) ----
                qt = qpool.tile([P, NPAIR, SB], bf16, tag="qt", name=f"qt{I}")
                for fc in range(2 * NPAIR):
                    pp = ps_proj.tile([P, SB], f32, tag="pp", name=f"pp{I}_{fc}")
                    for dc in range(NDC):
                        nc.tensor.matmul(
                            pp[:],
                            w_sb[:, dc, fc * P : (fc + 1) * P],
                            xt[:, dc, :],
                            start=(dc == 0),
                            stop=(dc == NDC - 1),
                        )
                    if I == 0 and fc < 4:
                        emit_warm(6)
                    tsh = tpool.tile([P, SB], f32, tag="tsh", name=f"tsh{I}_{fc}")
                    nc.vector.stream_shuffle(tsh[:], pp[:], mask=SWAP16)
                    dest = qt[:, fc, :] if fc < NPAIR else ktall[:, fc - NPAIR, scol]
                    nc.vector.tensor_tensor(dest, pp[:], cos_sb[:, scol], ALU.mult)
                    tsn = tpool.tile([P, SB], f32, tag="tsn", name=f"tsn{I}_{fc}")
                    nc.vector.tensor_tensor(tsn[:], tsh[:], sin_sb[:, scol], ALU.mult)
                    nc.vector.tensor_tensor(dest, dest, tsn[:], ALU.add)

                # ---- V projection ----
                for st in range(SB // P):
                    pp = ps_proj.tile([P, SB], f32, tag="pp", name=f"ppv{I}_{st}")
                    for dc in range(NDC):
                        nc.tensor.matmul(
                            pp[:],
                            xt[:, dc, st * P : (st + 1) * P],
                            w_sb[:, dc, 2 * DL : 3 * DL],
                            start=(dc == 0),
                            stop=(dc == NDC - 1),
                        )
                    nc.vector.tensor_copy(
                        v_sb[:, I * 4 + st, :, 0:DK],
                        pp[:].rearrange("p (h d) -> p h d", h=HL),
                    )

                if I == 0:
                    nc.sync.dma_start(wo_sb[:], wo_t[:])

                # ---- pipelined output projection of the PREVIOUS q block ----
                # Emitted here (between proj(I) and attention(I)) so the PE
                # has independent work at the attention(I-1) tail and at the
                # attention(I) head instead of stalling on DVE normalizes.
                if I > 0:
                    emit_outproj(I - 1, mh_prev)

                # ---- attention for q super-tile I ----
                # Per head pair: dense S^T phase (both heads row-tiled, one
                # paired exp per j over a 2-bank psum span), then dense AV
                # phase accumulating both heads.
                mhI = mpool.tile([P, NPAIR, SB], bf16, tag="mh", name=f"mh{I}")
                njt = 4 * I + 4
                for c in range(NPAIR):
                    pts = []
                    po0 = ps_av.tile([P, SB], f32, tag="po0", name=f"po{I}_{c}_0", bufs=1)
                    po1 = ps_av.tile([P, SB], f32, tag="po1", name=f"po{I}_{c}_1", bufs=1)
                    for j in range(njt):
                        m = j - 4 * I
                        off = m * P if m > 0 else 0
                        N = SB - off
                        pss = ps_s.tile([P, 2, SB], f32, tag="pss", name=f"pss{I}_{c}_{j}")
                        for half in (0, 1):
                            pr = 64 * half
                            nc.tensor.matmul(
                                pss[:, half, :N],
                                ktall[pr : pr + 64, c, j * P : (j + 1) * P],
                                qt[pr : pr + 64, c, off:SB],
                                start=True,
                                stop=True,
                            )
                        pt = ptpool.tile(
                            [P, 2, SB], bf16, tag="pt", name=f"pt{I}_{c}_{j}", bufs=NST
                        )
                        nc.scalar.activation(
                            pt[:, :, :N], pss[:, :, :N], FX.Exp, scale=0.125
                        )
                        if m >= 0:
                            nc.vector.tensor_tensor(
                                pt[:, :, 0:P], pt[:, :, 0:P],
                                mask_sb[:, None, :].to_broadcast((P, 2, P)),
                                ALU.mult,
                            )
                        pts.append((pt, off, N))
                    for j in range(njt):
                        pt, off, N = pts[j]
                        for half, po in ((0, po0), (1, po1)):
                            nc.tensor.matmul(
                                po[:, off:SB],
                                v_sb[:, j, 2 * c + half, :],
                                pt[:, half, :N],
                                start=(j == 0),
                                stop=(j == njt - 1),
                                skip_group_check=True,
                            )
                    for half, po in ((0, po0), (1, po1)):
                        pr = 64 * half
                        lsb = spool.tile([64, SB], f32, tag="lsb", name=f"lsb{I}_{c}_{half}")
                        nc.vector.tensor_copy(lsb[:], po[DK:P, :])
                        rec = spool.tile([64, SB], f32, tag="rec", name=f"rec{I}_{c}_{half}")
                        nc.vector.reciprocal_approx_fast(rec[:], lsb[:])
                        nc.vector.tensor_tensor(
                            mhI[pr : pr + 64, c, :], po[0:DK, :], rec[:], ALU.mult
                        )

                mh_prev = mhI
            emit_outproj(NSB - 1, mh_prev)
    nc.finalize()
    return nc


_NC = None


def _get_nc():
    global _NC
    if _NC is None:
        _NC = _build()
    return _NC


def _host_prep(Wq, Wk, Wv, Wo):
    t = np.arange(DK // 2)
    qd, rd = t // 16, t % 16
    perm = np.empty(DK, np.int64)
    perm[qd * 32 + rd] = 2 * t
    perm[qd * 32 + 16 + rd] = 2 * t + 1

    Wq_p = Wq.reshape(HEADS, DK, D)[:, perm, :].reshape(HEADS * DK, D)
    Wk_p = Wk.reshape(HEADS, DK, D)[:, perm, :].reshape(HEADS * DK, D)

    pos = np.arange(S, dtype=np.float64)
    inv = 1.0 / THETA ** (np.arange(0, DK, 2).astype(np.float64) / DK)  # [32]
    ang = inv[:, None] * pos[None, :]                                   # [32, S]
    cos32 = np.cos(ang).astype(np.float32)
    sin32 = np.sin(ang).astype(np.float32)
    cosf = np.empty((DK, S), np.float32)
    sinf = np.empty((DK, S), np.float32)
    rows_lo = qd * 32 + rd
    rows_hi = qd * 32 + 16 + rd
    cosf[rows_lo] = cos32[t]
    cosf[rows_hi] = cos32[t]
    sinf[rows_lo] = -sin32[t]
    sinf[rows_hi] = sin32[t]

    mask01 = (
        np.arange(P)[:, None] <= np.arange(P)[None, :]
    ).astype(ml_dtypes.bfloat16)

    per_tp = []
    for tp in range(TP):
        sl = slice(tp * DL, (tp + 1) * DL)
        wqkvT = np.ascontiguousarray(
            np.concatenate([Wq_p[sl], Wk_p[sl], Wv[sl]], axis=0).T
        ).astype(ml_dtypes.bfloat16)
        woT = np.ascontiguousarray(Wo[:, sl].T).astype(ml_dtypes.bfloat16)
        per_tp.append((wqkvT, woT))
    return per_tp, cosf, sinf, mask01


def kernel(x, Wq, Wk, Wv, Wo):
    x = np.asarray(x, np.float32)
    Wq = np.asarray(Wq, np.float32)
    Wk = np.asarray(Wk, np.float32)
    Wv = np.asarray(Wv, np.float32)
    Wo = np.asarray(Wo, np.float32)

    per_tp, cosf, sinf, mask01 = _host_prep(Wq, Wk, Wv, Wo)
    xTs = [np.ascontiguousarray(x[b].T).astype(ml_dtypes.bfloat16) for b in range(B)]

    in_maps = []
    for core in range(DP * TP):
        b, tp = core // TP, core % TP
        wqkvT, woT = per_tp[tp]
        in_maps.append(
            {
                "xT": xTs[b],
                "wqkvT": wqkvT,
                "woT": woT,
                "cosf": cosf,
                "sinf": sinf,
                "maskt": mask01,
            }
        )

    nc = _get_nc()
    res = run_bass_kernel_spmd(nc, in_maps, core_ids=list(range(DP * TP)))
    out = np.empty((B, S, D), np.float32)
    for b in range(B):
        out[b] = res.results[b * TP]["out"] + res.results[b * TP + 1]["out"]
    return out


# revision 14
# speedup vs baseline: 1.0311x; 1.0311x over previous
"""MHSA (RoPE + causal softmax) Trainium2 Bass kernel.

Problem: x[4,2048,1024], Wq/Wk/Wv/Wo[1024,1024] fp32; 16 heads, d_k=64.

Sharding over the 8 NeuronCores: 4-way data-parallel over batch x 2-way
tensor-parallel over heads. core = 2*b + t handles batch b, heads
[t*8, t*8+8). Column-parallel Wq/Wk/Wv, row-parallel Wo; the two TP
partial outputs per batch are summed on the host (the gather step).

Device-side design (per core, all sizes hardcoded):
  - Host supplies x^T [1024,2048] so every matmul contraction dim (model
    dim d or k-positions) lands on SBUF partitions. float32r matmuls
    (fp32 bit layout, tf32-like multiply) run at ~1 cycle/row.
  - RoPE: interleaved even/odd pairs are pre-permuted in the Wq/Wk ROWS
    (host side) so each rotation partner lives 16 partitions away within
    a 32-partition quadrant; the rotation is then
        q' = q * cosT + stream_shuffle(q) * sinT
    with a single DVE stream-shuffle (swap 16-blocks) and sign baked
    into sinT. Scores are permutation-invariant so nothing downstream
    changes.
  - Attention per head pair (2 heads stacked on 128 partitions,
    auto row-tiled matmuls at base partitions 0/64):
       S^T[j] = K_j @ Q_I^T          (f32r, [128 kpos, <=512 q])
       P^T    = exp(S^T / 8)          (ACT, psum -> bf16 sbuf)
       causal: block-skip j>4I+3, narrow diagonal tiles, one [128,128]
       tri-mask multiply on the diagonal block
       O^T   += [V_j | 1]^T @ P^T     (bf16, gives output AND softmax
                                       denominator in row 64 - no
                                       max-subtraction softmax, safe for
                                       these magnitudes)
       mh^T   = O^T[0:64] * bcast(1/O^T[64])
  - Output projection back in [s, d] layout (lhsT = mh^T tiles) so the
    DRAM write is contiguous; host sums the two TP partials per batch.
"""
import numpy as np
import ml_dtypes

import concourse.bass as bass
from concourse import bacc
import concourse.tile as tile
import concourse.mybir as mybir
from concourse.bass_utils import run_bass_kernel_spmd

B, S, D = 4, 2048, 1024
HEADS, DK = 16, 64
THETA = 10000.0
TP, DP = 2, 4
HL = HEADS // TP            # 8 local heads per core
DL = HL * DK                # 512 local projection width
P = 128
SB = 512                    # q super-tile width
NSB = S // SB               # 4 q super-tiles (I)
NST = S // P                # 16 k-tiles (j)
NDC = D // P                # 8 contraction chunks over model dim
NPAIR = HL // 2             # 4 local head pairs

f32 = mybir.dt.float32
f32r = mybir.dt.float32r
bf16 = mybir.dt.bfloat16
SWAP16 = [(i + 16) % 32 for i in range(32)]


def _build():
    ALU = mybir.AluOpType
    FX = mybir.ActivationFunctionType
    nc = bacc.Bacc(None, target_bir_lowering=False)

    xT = nc.dram_tensor("xT", [D, S], bf16, kind="ExternalInput")
    wqkvT = nc.dram_tensor("wqkvT", [D, 3 * DL], bf16, kind="ExternalInput")
    woT = nc.dram_tensor("woT", [DL, D], bf16, kind="ExternalInput")
    cosf = nc.dram_tensor("cosf", [DK, S], f32, kind="ExternalInput")
    sinf = nc.dram_tensor("sinf", [DK, S], f32, kind="ExternalInput")
    maskt = nc.dram_tensor("maskt", [P, P], bf16, kind="ExternalInput")
    out = nc.dram_tensor("out", [S, D], f32, kind="ExternalOutput")

    xT_t = xT.rearrange("(dc p) s -> p dc s", p=P)        # [128, 8, 2048]
    wq_t = wqkvT.rearrange("(dc p) f -> p dc f", p=P)     # [128, 8, 1536]
    wo_t = woT.rearrange("(c p) f -> p c f", p=P)         # [128, 4, 1024]

    with tile.TileContext(nc) as tc:
        with (
            tc.tile_pool(name="wpool", bufs=1) as wpool,
            tc.tile_pool(name="kpool", bufs=1) as kpool,
            tc.tile_pool(name="xpool", bufs=2) as xpool,
            tc.tile_pool(name="qpool", bufs=2) as qpool,
            tc.tile_pool(name="tpool", bufs=2) as tpool,
            tc.tile_pool(name="ptpool", bufs=4) as ptpool,
            tc.tile_pool(name="mpool", bufs=2) as mpool,
            tc.tile_pool(name="spool", bufs=2) as spool,
            tc.tile_pool(name="opool", bufs=2) as opool,
            tc.tile_pool(name="ps_proj", bufs=2, space="PSUM") as ps_proj,
            tc.tile_pool(name="ps_s", bufs=2, space="PSUM") as ps_s,
            tc.tile_pool(name="ps_av", bufs=1, space="PSUM") as ps_av,
        ):
            # First q-block x chunks go out first so the PE can start within
            # a few us; bulky constants stream behind them.
            xt0 = xpool.tile([P, NDC, SB], bf16, tag="xt", name="xt0")
            w_sb = wpool.tile([P, NDC, 3 * DL], bf16)
            for dc in range(NDC):
                eng = nc.sync if dc % 2 == 0 else nc.scalar
                eng.dma_start(xt0[:, dc, :], xT_t[:, dc, 0:SB])
                eng2 = nc.scalar if dc % 2 == 0 else nc.sync
                eng2.dma_start(w_sb[:, dc, 0:DL], wq_t[:, dc, 0:DL])
            for dc in range(NDC):
                eng = nc.sync if dc % 2 == 0 else nc.scalar
                eng.dma_start(w_sb[:, dc, DL : 3 * DL], wq_t[:, dc, DL : 3 * DL])
            cos_sb = wpool.tile([P, S], f32)
            sin_sb = wpool.tile([P, S], f32)
            nc.gpsimd.dma_start(cos_sb[0:DK, :], cosf[:])
            nc.gpsimd.dma_start(cos_sb[DK:P, :], cosf[:])
            nc.gpsimd.dma_start(sin_sb[0:DK, :], sinf[:])
            nc.gpsimd.dma_start(sin_sb[DK:P, :], sinf[:])
            mask_sb = wpool.tile([P, P], bf16)
            nc.gpsimd.dma_start(mask_sb[:], maskt[:])
            wo_sb = wpool.tile([P, DL // P, D], bf16)

            ktall = kpool.tile([P, NPAIR, S], bf16)
            v_sb = kpool.tile([P, NST, HL, 2 * DK], bf16)
            nc.gpsimd.memset(v_sb[:, :, :, DK : 2 * DK], 1.0)

            # PE warm-up: the first ~15us are DMA-bound; keep the tensor
            # engine active so the HAM clock gate reaches (and holds) 2.4GHz
            # before the real matmuls arrive.
            warm = wpool.tile([P, SB], bf16)
            nc.gpsimd.memset(warm[:], 0.0)
            warm_sink = wpool.tile([P, SB], f32)

            def emit_warm(n):
                pw = ps_s.tile([P, 2, SB], f32, tag="pss", name=f"warm{emit_warm.k}")
                emit_warm.k += 1
                for r in range(n):
                    nc.tensor.matmul(pw[:, 0, :], warm[:, 0:P], warm[:],
                                     start=True, stop=True)
                nc.vector.tensor_copy(warm_sink[:], pw[:, 0, :])
            emit_warm.k = 0
            emit_warm(16)

            def emit_outproj(Io, mh):
                # row-parallel partial output projection for q block Io
                for st in range(SB // P):
                    osb = opool.tile([P, D], f32, tag="osb", name=f"osb{Io}_{st}")
                    for oh in (0, 1):
                        pq = ps_proj.tile([P, SB], f32, tag="pp", name=f"pq{Io}_{st}_{oh}")
                        for c2 in range(NPAIR):
                            nc.tensor.matmul(
                                pq[:],
                                mh[:, c2, st * P : (st + 1) * P],
                                wo_sb[:, c2, oh * SB : (oh + 1) * SB],
                                start=(c2 == 0),
                                stop=(c2 == NPAIR - 1),
                            )
                        nc.vector.tensor_copy(osb[:, oh * SB : (oh + 1) * SB], pq[:])
                    nc.sync.dma_start(
                        out[(Io * 4 + st) * P : (Io * 4 + st + 1) * P, :], osb[:]
                    )

            mh_prev = None
            for I in range(NSB):
                scol = slice(I * SB, (I + 1) * SB)
                if I == 0:
                    xt = xt0
                else:
                    xt = xpool.tile([P, NDC, SB], bf16, tag="xt", name=f"xt{I}")
                    for dc in range(NDC):
                        eng = nc.sync if dc % 2 == 0 else nc.scalar
                        eng.dma_start(xt[:, dc, :], xT_t[:, dc, scol])

                # ---- Q/K projections + RoPE (f-chunk = head pair) ----
                qt = qpool.tile([P, NPAIR, SB], bf16, tag="qt", name=f"qt{I}")
                for fc in range(2 * NPAIR):
                    pp = ps_proj.tile([P, SB], f32, tag="pp", name=f"pp{I}_{fc}")
                    for dc in range(NDC):
                        nc.tensor.matmul(
                            pp[:],
                            w_sb[:, dc, fc * P : (fc + 1) * P],
                            xt[:, dc, :],
                            start=(dc == 0),
                            stop=(dc == NDC - 1),
                        )
                    tsh = tpool.tile([P, SB], f32, tag="tsh", name=f"tsh{I}_{fc}")
                    nc.vector.stream_shuffle(tsh[:], pp[:], mask=SWAP16)
                    dest = qt[:, fc, :] if fc < NPAIR else ktall[:, fc - NPAIR, scol]
                    nc.vector.tensor_tensor(dest, pp[:], cos_sb[:, scol], ALU.mult)
                    tsn = tpool.tile([P, SB], f32, tag="tsn", name=f"tsn{I}_{fc}")
                    nc.vector.tensor_tensor(tsn[:], tsh[:], sin_sb[:, scol], ALU.mult)
                    nc.vector.tensor_tensor(dest, dest, tsn[:], ALU.add)

                # ---- V projection ----
                for st in range(SB // P):
                    pp = ps_proj.tile([P, SB], f32, tag="pp", name=f"ppv{I}_{st}")
                    for dc in range(NDC):
                        nc.tensor.matmul(
                            pp[:],
                            xt[:, dc, st * P : (st + 1) * P],
                            w_sb[:, dc, 2 * DL : 3 * DL],
                            start=(dc == 0),
                            stop=(dc == NDC - 1),
                        )
                    nc.vector.tensor_copy(
                        v_sb[:, I * 4 + st, :, 0:DK],
                        pp[:].rearrange("p (h d) -> p h d", h=HL),
                    )

                if I == 0:
                    nc.sync.dma_start(wo_sb[:], wo_t[:])

                # ---- pipelined output projection of the PREVIOUS q block ----
                # Emitted here (between proj(I) and attention(I)) so the PE
                # has independent work at the attention(I-1) tail and at the
                # attention(I) head instead of stalling on DVE normalizes.
                if I > 0:
                    emit_outproj(I - 1, mh_prev)

                # ---- attention for q super-tile I ----
                # Per head pair: dense S^T phase (both heads row-tiled, one
                # paired exp per j over a 2-bank psum span), then dense AV
                # phase accumulating both heads.
                mhI = mpool.tile([P, NPAIR, SB], bf16, tag="mh", name=f"mh{I}")
                njt = 4 * I + 4
                for c in range(NPAIR):
                    pts = []
                    po0 = ps_av.tile([P, SB], f32, tag="po0", name=f"po{I}_{c}_0", bufs=1)
                    po1 = ps_av.tile([P, SB], f32, tag="po1", name=f"po{I}_{c}_1", bufs=1)
                    for j in range(njt):
                        m = j - 4 * I
                        off = m * P if m > 0 else 0
                        N = SB - off
                        pss = ps_s.tile([P, 2, SB], f32, tag="pss", name=f"pss{I}_{c}_{j}")
                        for half in (0, 1):
                            pr = 64 * half
                            nc.tensor.matmul(
                                pss[:, half, :N],
                                ktall[pr : pr + 64, c, j * P : (j + 1) * P],
                                qt[pr : pr + 64, c, off:SB],
                                start=True,
                                stop=True,
                            )
                        pt = ptpool.tile(
                            [P, 2, SB], bf16, tag="pt", name=f"pt{I}_{c}_{j}", bufs=NST
                        )
                        nc.scalar.activation(
                            pt[:, :, :N], pss[:, :, :N], FX.Exp, scale=0.125
                        )
                        if m >= 0:
                            nc.vector.tensor_tensor(
                                pt[:, :, 0:P], pt[:, :, 0:P],
                                mask_sb[:, None, :].to_broadcast((P, 2, P)),
                                ALU.mult,
                            )
                        pts.append((pt, off, N))
                    for j in range(njt):
                        pt, off, N = pts[j]
                        for half, po in ((0, po0), (1, po1)):
                            nc.tensor.matmul(
                                po[:, off:SB],
                                v_sb[:, j, 2 * c + half, :],
                                pt[:, half, :N],
                                start=(j == 0),
                                stop=(j == njt - 1),
                                skip_group_check=True,
                            )
                    for half, po in ((0, po0), (1, po1)):
                        pr = 64 * half
                        lsb = spool.tile([64, SB], f32, tag="lsb", name=f"lsb{I}_{c}_{half}")
                        nc.vector.tensor_copy(lsb[:], po[DK:P, :])
                        rec = spool.tile([64, SB], f32, tag="rec", name=f"rec{I}_{c}_{half}")
                        nc.vector.reciprocal_approx_fast(rec[:], lsb[:])
                        nc.vector.tensor_tensor(
                            mhI[pr : pr + 64, c, :], po[0:DK, :], rec[:], ALU.mult
                        )

                mh_prev = mhI
            emit_outproj(NSB - 1, mh_prev)
    nc.finalize()
    return nc


_NC = None


def _get_nc():
    global _NC
    if _NC is None:
        _NC = _build()
    return _NC


def _host_prep(Wq, Wk, Wv, Wo):
    t = np.arange(DK // 2)
    qd, rd = t // 16, t % 16
    perm = np.empty(DK, np.int64)
    perm[qd * 32 + rd] = 2 * t
    perm[qd * 32 + 16 + rd] = 2 * t + 1

    Wq_p = Wq.reshape(HEADS, DK, D)[:, perm, :].reshape(HEADS * DK, D)
    Wk_p = Wk.reshape(HEADS, DK, D)[:, perm, :].reshape(HEADS * DK, D)

    pos = np.arange(S, dtype=np.float64)
    inv = 1.0 / THETA ** (np.arange(0, DK, 2).astype(np.float64) / DK)  # [32]
    ang = inv[:, None] * pos[None, :]                                   # [32, S]
    cos32 = np.cos(ang).astype(np.float32)
    sin32 = np.sin(ang).astype(np.float32)
    cosf = np.empty((DK, S), np.float32)
    sinf = np.empty((DK, S), np.float32)
    rows_lo = qd * 32 + rd
    rows_hi = qd * 32 + 16 + rd
    cosf[rows_lo] = cos32[t]
    cosf[rows_hi] = cos32[t]
    sinf[rows_lo] = -sin32[t]
    sinf[rows_hi] = sin32[t]

    mask01 = (
        np.arange(P)[:, None] <= np.arange(P)[None, :]
    ).astype(ml_dtypes.bfloat16)

    per_tp = []
    for tp in range(TP):
        sl = slice(tp * DL, (tp + 1) * DL)
        wqkvT = np.ascontiguousarray(
            np.concatenate([Wq_p[sl], Wk_p[sl], Wv[sl]], axis=0).T
        ).astype(ml_dtypes.bfloat16)
        woT = np.ascontiguousarray(Wo[:, sl].T).astype(ml_dtypes.bfloat16)
        per_tp.append((wqkvT, woT))
    return per_tp, cosf, sinf, mask01


def kernel(x, Wq, Wk, Wv, Wo):
    x = np.asarray(x, np.float32)
    Wq = np.asarray(Wq, np.float32)
    Wk = np.asarray(Wk, np.float32)
    Wv = np.asarray(Wv, np.float32)
    Wo = np.asarray(Wo, np.float32)

    per_tp, cosf, sinf, mask01 = _host_prep(Wq, Wk, Wv, Wo)
    xTs = [np.ascontiguousarray(x[b].T).astype(ml_dtypes.bfloat16) for b in range(B)]

    in_maps = []
    for core in range(DP * TP):
        b, tp = core // TP, core % TP
        wqkvT, woT = per_tp[tp]
        in_maps.append(
            {
                "xT": xTs[b],
                "wqkvT": wqkvT,
                "woT": woT,
                "cosf": cosf,
                "sinf": sinf,
                "maskt": mask01,
            }
        )

    nc = _get_nc()
    res = run_bass_kernel_spmd(nc, in_maps, core_ids=list(range(DP * TP)))
    out = np.empty((B, S, D), np.float32)
    for b in range(B):
        out[b] = res.results[b * TP]["out"] + res.results[b * TP + 1]["out"]
    return out
